# revision 1
# baseline (speedup 1.0000x reference)
"""Trainium2 Bass kernel for nn_ContextQueryAttention (B=64, H=128, C=1024, Q=128).

Sharding: pure data-parallel over batch — 8 batches per NeuronCore, SPMD on 8
cores. Params (tiny H-vectors) replicated to every core.

Math (masks are all-ones, so masked softmax == plain softmax; softmax shift
invariance lets each score layout carry only its per-partition-friendly bias):
  S = s0[c] + s1[q] + s2[c,q] + bias,  s2 = (c*cqw)^T q  (contraction over H)
  a_att = softmax_q(S): independent of s0/bias;  computed from ET = exp(s2^T + s1)
  b_att = softmax_c(S): independent of s1/bias;  computed from Ec = exp(s2 + s0)
  a^T = q^T @ A_T,     A_T = ET / colsum(ET)                 [H,C]
  tmp = Ec^T @ c^T,    tmp2 = tmp / db,  db = colsum_c(Ec)   [Q,H]
  b^T = tmp2^T @ A_T                                          [H,C]
  out[b] = rows [c; a^T; c*a^T; c*b^T]                        [4H, C]

Matmuls run in bf16 (fp32 PSUM accumulation); exp/normalizers in fp32.
"""

import numpy as np
from contextlib import ExitStack

import concourse.bass as bass
import concourse.bacc as bacc
import concourse.tile as tile
from concourse import mybir
from concourse.bass_utils import run_bass_kernel_spmd
from concourse.masks import make_identity

F32 = mybir.dt.float32
BF16 = mybir.dt.bfloat16
EXP = mybir.ActivationFunctionType.Exp
COPY = mybir.ActivationFunctionType.Copy

B, H, C, Q = 64, 128, 1024, 128
NCORES = 8
NB = B // NCORES  # batches per core
NCK = C // 128    # 8 column chunks of C


def _body(ctx: ExitStack, tc: tile.TileContext, c_in, q_in, ctxw_in, qw_in,
          cqw_in, out, nb: int):
    nc = tc.nc

    const = ctx.enter_context(tc.tile_pool(name="const", bufs=1))
    big = ctx.enter_context(tc.tile_pool(name="big", bufs=4))
    poolc = ctx.enter_context(tc.tile_pool(name="poolc", bufs=8))
    poolo = ctx.enter_context(tc.tile_pool(name="poolo", bufs=4))
    med = ctx.enter_context(tc.tile_pool(name="med", bufs=4))
    small = ctx.enter_context(tc.tile_pool(name="small", bufs=4))
    # PSUM budget (8 banks): psA 4 (shared 2KB slots) + psCT 2 + psMisc 2
    psA = ctx.enter_context(tc.tile_pool(name="psA", bufs=4, space="PSUM"))
    psCT = ctx.enter_context(tc.tile_pool(name="psCT", bufs=2, space="PSUM"))
    psMisc = ctx.enter_context(tc.tile_pool(name="psM", bufs=2, space="PSUM"))

    # --- per-core constants ---
    ident_f = const.tile([128, 128], F32)
    make_identity(nc, ident_f)
    ident_b = const.tile([128, 128], BF16)
    make_identity(nc, ident_b)
    ones_b = const.tile([128, 128], BF16)
    nc.vector.memset(ones_b, 1.0)
    ctxw = const.tile([128, 1], F32)
    nc.gpsimd.dma_start(ctxw, ctxw_in[:, :])
    qw = const.tile([128, 1], F32)
    nc.gpsimd.dma_start(qw, qw_in[:, :])
    cqw = const.tile([128, 1], F32)
    nc.gpsimd.dma_start(cqw, cqw_in[:, :])
    rcqw = const.tile([128, 1], F32)
    nc.vector.reciprocal(rcqw, cqw)

    for b in range(nb):
        # ---- loads; the c row-block of the output is written back as soon
        # as it lands so the out-DMA stream starts early ----
        c_sb = poolc.tile([128, C], F32, tag="c_sb")
        nc.sync.dma_start(c_sb, c_in[b])
        q_sb = med.tile([128, Q], F32, tag="q_sb")
        nc.sync.dma_start(q_sb, q_in[b])
        nc.sync.dma_start(out[b, 0:128, :], c_sb)
        # out3 holds the computed row-blocks [aT; c*aT; c*bT]
        out3 = poolo.tile([128, 3, C], F32, tag="out3")

        # ---- casts / scaled copies ----
        c_scaled = big.tile([128, C], BF16, tag="c_scaled")   # (c * cqw) in bf16
        nc.vector.tensor_scalar_mul(c_scaled, c_sb, cqw)
        q_bf = med.tile([128, Q], BF16, tag="q_bf")
        nc.vector.tensor_copy(q_bf, q_sb)

        # ---- misc PSUM scratch (single bank) ----
        misc = psMisc.tile([128, 260], F32, tag="misc")
        s1_ps = misc[:, 0:1]
        s0_ps = misc[:, 1:9]
        tmpdb_ps = misc[:, 128:257]   # tmp in [:,0:128], db in [:,128]
        tmp_ps = tmpdb_ps[:, 0:128]
        db_ps = tmpdb_ps[:, 128:129]

        # ---- s1[q] = sum_h q[h,q]*qw[h] (fp32, N=1) ----
        nc.tensor.matmul(s1_ps, q_sb, qw)
        s1_sb = small.tile([128, 1], F32, tag="s1")
        nc.vector.tensor_copy(s1_sb, s1_ps)

        # ---- qT via PE transpose (fp32), evac-cast to bf16 ----
        qT_ps = psA.tile([128, 128], F32, tag="psA")
        nc.tensor.transpose(qT_ps, q_sb, ident_f)
        qT_bf = small.tile([128, 128], BF16, tag="qT")
        nc.vector.tensor_copy(qT_bf, qT_ps)

        # ---- S_T halves + ET = exp(S_T + s1) ----
        ET = big.tile([128, C], BF16, tag="ET")
        for h2 in range(2):
            sl = slice(512 * h2, 512 * (h2 + 1))
            st = psA.tile([128, 512], F32, tag="psA")
            nc.tensor.matmul(st, q_bf, c_scaled[:, sl])
            nc.scalar.activation(ET[:, sl], st, EXP, bias=s1_sb, scale=1.0)

        # ---- s0 chunks + S chunks; Ec = exp(S_c) (es0 applied via cT) ----
        Ec = big.tile([128, NCK, 128], BF16, tag="Ec")
        for half in range(2):
            sc = psA.tile([128, 4, 128], F32, tag="psA")
            for j4 in range(4):
                j = half * 4 + j4
                csl = slice(128 * j, 128 * (j + 1))
                nc.tensor.matmul(s0_ps[:, j:j + 1], c_sb[:, csl], ctxw)
                nc.tensor.matmul(sc[:, j4, :], c_scaled[:, csl], q_bf)
            nc.scalar.activation(Ec[:, 4 * half:4 * half + 4, :], sc, EXP)

        # cT carries es0[c] (exp of s0, per-partition) and the cqw[h] scale
        # from c_scaled; col 128 holds es0 itself so the tmp matmul also
        # accumulates db = sum_c Ec*es0 in its last output column.
        cT = big.tile([128, NCK, 129], BF16, tag="cT")
        es0 = small.tile([128, 8], F32, tag="es0")
        nc.scalar.activation(es0, s0_ps, EXP)
        nc.scalar.activation(cT[:, :, 128:129], es0, COPY)
        for half in range(2):
            ct_ps = psCT.tile([128, 4, 128], BF16, tag="ct")
            for j4 in range(4):
                j = half * 4 + j4
                nc.tensor.transpose(ct_ps[:, j4, :],
                                    c_scaled[:, 128 * j:128 * (j + 1)], ident_b)
            for j4 in range(4):
                j = half * 4 + j4
                nc.vector.tensor_scalar_mul(cT[:, j, 0:128], ct_ps[:, j4, :],
                                            es0[:, j:j + 1])

        # ---- D_A = colsum(ET) broadcast; recD = 1/D_A; A_T = ET*recD ----
        recD = big.tile([128, C], F32, tag="recD")
        for h2 in range(2):
            sl = slice(512 * h2, 512 * (h2 + 1))
            da = psA.tile([128, 512], F32, tag="psA")
            nc.tensor.matmul(da, ones_b, ET[:, sl])
            nc.vector.reciprocal(recD[:, sl], da)
        A_T = big.tile([128, C], BF16, tag="A_T")
        nc.vector.tensor_mul(A_T[:, 0:512], ET[:, 0:512], recD[:, 0:512])
        nc.gpsimd.tensor_mul(A_T[:, 512:], ET[:, 512:], recD[:, 512:])

        # ---- [tmp | db] = sum_j Ec_j^T @ [cs0T_j | es0_j] (fused, one group) ----
        for j in range(NCK):
            nc.tensor.matmul(tmpdb_ps, Ec[:, j, :], cT[:, j, :],
                             start=(j == 0), stop=(j == NCK - 1))
        rdb = small.tile([128, 1], F32, tag="rdb")
        nc.vector.reciprocal(rdb, db_ps)
        tmp2 = small.tile([128, 128], BF16, tag="tmp2")
        nc.vector.tensor_scalar_mul(tmp2, tmp_ps, rdb)

        # ---- aT = qT^T @ A_T ; bT = (1/cqw) * (tmp2^T @ A_T) (halves) ----
        aT_sb = out3[:, 0, :]
        bT_sb = big.tile([128, C], F32, tag="bT_sb")
        for h2 in range(2):
            sl = slice(512 * h2, 512 * (h2 + 1))
            ap = psA.tile([128, 512], F32, tag="psA")
            nc.tensor.matmul(ap, qT_bf, A_T[:, sl])
            nc.scalar.activation(aT_sb[:, sl], ap, COPY)
        for h2 in range(2):
            sl = slice(512 * h2, 512 * (h2 + 1))
            bp = psA.tile([128, 512], F32, tag="psA")
            nc.tensor.matmul(bp, tmp2, A_T[:, sl])
            nc.scalar.activation(bT_sb[:, sl], bp, COPY, scale=rcqw)

        # ---- elementwise products (ca on Pool ∥ cb on DVE; cb is the
        # later product, so it gets the faster engine) ----
        nc.gpsimd.tensor_mul(out3[:, 1, :], c_sb, aT_sb)
        nc.vector.tensor_mul(out3[:, 2, :], c_sb, bT_sb)

        # ---- store: one DMA for the 3 computed row-blocks ----
        nc.sync.dma_start(
            out[b, 128:512, :].rearrange("(k h) c -> h k c", h=128), out3)


def build_nc(nb: int = NB) -> bass.Bass:
    nc = bacc.Bacc("TRN2", target_bir_lowering=False, debug=False)
    c_in = nc.declare_dram_parameter("c", [nb, H, C], F32, isOutput=False)
    q_in = nc.declare_dram_parameter("q", [nb, H, Q], F32, isOutput=False)
    ctxw = nc.declare_dram_parameter("ctxw", [H, 1], F32, isOutput=False)
    qw = nc.declare_dram_parameter("qw", [H, 1], F32, isOutput=False)
    cqw = nc.declare_dram_parameter("cqw", [H, 1], F32, isOutput=False)
    out = nc.declare_dram_parameter("out", [nb, 4 * H, C], F32, isOutput=True)
    with tile.TileContext(nc) as tc:
        with ExitStack() as ctx:
            _body(ctx, tc, c_in[:], q_in[:], ctxw[:], qw[:], cqw[:], out[:], nb)
    nc.compile()
    return nc


_NC_CACHE: dict = {}


def _get_nc(nb: int) -> bass.Bass:
    if nb not in _NC_CACHE:
        _NC_CACHE[nb] = build_nc(nb)
    return _NC_CACHE[nb]


def make_in_maps(inputs: dict, ncores: int = NCORES):
    c = np.ascontiguousarray(np.asarray(inputs["c"], dtype=np.float32))
    q = np.ascontiguousarray(np.asarray(inputs["q"], dtype=np.float32))
    ctxw = np.ascontiguousarray(
        np.asarray(inputs["context_weights"], np.float32).reshape(H, 1))
    qw = np.ascontiguousarray(
        np.asarray(inputs["query_weights"], np.float32).reshape(H, 1))
    cqw = np.ascontiguousarray(
        np.asarray(inputs["cq_weights"], np.float32).reshape(H, 1))
    nb = c.shape[0] // ncores
    return [
        {
            "c": c[i * nb:(i + 1) * nb],
            "q": q[i * nb:(i + 1) * nb],
            "ctxw": ctxw,
            "qw": qw,
            "cqw": cqw,
        }
        for i in range(ncores)
    ], nb


def kernel(**inputs) -> np.ndarray:
    in_maps, nb = make_in_maps(inputs)
    nc = _get_nc(nb)
    res = run_bass_kernel_spmd(nc, in_maps, list(range(NCORES)))
    return np.concatenate([res.results[i]["out"] for i in range(NCORES)], axis=0)



# revision 5
# speedup vs baseline: 1.2312x; 1.2312x over previous
"""Trainium2 Bass kernel for nn_ContextQueryAttention (B=64, H=128, C=1024, Q=128).

Sharding: pure data-parallel over batch — 8 batches per NeuronCore, SPMD on 8
cores. Params (tiny H-vectors) replicated to every core.

Math (masks are all-ones; softmax shift invariance lets bias be dropped):
  S = s0[c] + s1[q] + s2[c,q],  s2 = (c*cqw)^T q  (contraction over H)
  Fold s0 into the score matmul:  q_cs[h,q] = q[h,q]*cqw[h] + ctxw[h]
    => q_cs^T @ c = s2^T + s0  (row-broadcast), so ET = exp(S^T + s1) fully.
  a_att = softmax_q(S):  A_T = ET / colsum_q(ET)            [q, C]
  a^T   = qT^T @ A_T                                        [h, C]
  Ec    = transpose(ET) = exp(S) chunks                     [c, q]
  [tmp | db] = sum_j Ec_j^T @ [cT_j | 1]  (db = colsum_c)   [q, h+1]
  tmp2  = tmp / db;  b^T = tmp2^T @ A_T                     [h, C]
  out rows = [c; a^T; c*a^T; c*b^T]  — the c block is an identity copy of the
  input and is assembled host-side during the gather; device emits the other
  three row-blocks in bf16.

Host pre-shards c in bf16 and also ships q transposed (bf16) so the device
needs no q transpose. All matmuls bf16 with f32 PSUM.
"""

import numpy as np
import ml_dtypes
from contextlib import ExitStack

import concourse.bass as bass
import concourse.bacc as bacc
import concourse.tile as tile
from concourse import mybir
from concourse.bass_utils import run_bass_kernel_spmd
from concourse.masks import make_identity

F32 = mybir.dt.float32
BF16 = mybir.dt.bfloat16
EXP = mybir.ActivationFunctionType.Exp
COPY = mybir.ActivationFunctionType.Copy
MULT = mybir.AluOpType.mult
ADD = mybir.AluOpType.add
DIV = mybir.AluOpType.divide

B, H, C, Q = 64, 128, 1024, 128
NCORES = 8
NB = B // NCORES  # batches per core
NCK = C // 128    # 8 column chunks of C

# normalize a_att via DVE tensor_tensor divide straight from PSUM; fallback
# is reciprocal into bf16 recD plus a 4x multiply
USE_DIVIDE = False


def _body(ctx: ExitStack, tc: tile.TileContext, c_in, q_in, qT_in, ctxw_in,
          qw_in, cqw_in, out_a, out_cc, nb: int):
    nc = tc.nc

    const = ctx.enter_context(tc.tile_pool(name="const", bufs=1))
    big = ctx.enter_context(tc.tile_pool(name="big", bufs=2))
    poolc = ctx.enter_context(tc.tile_pool(name="poolc", bufs=3))
    small = ctx.enter_context(tc.tile_pool(name="small", bufs=2))
    # PSUM (8 banks): psA 4 x [128,512]f32 + psT 2 x [128,8,128]bf16 + psM 2
    psA = ctx.enter_context(tc.tile_pool(name="psA", bufs=4, space="PSUM"))
    psT = ctx.enter_context(tc.tile_pool(name="psT", bufs=2, space="PSUM"))
    psM = ctx.enter_context(tc.tile_pool(name="psM", bufs=2, space="PSUM"))

    # --- per-core constants ---
    ident_b = const.tile([128, 128], BF16)
    make_identity(nc, ident_b)
    ones_b = const.tile([128, 128], BF16)
    nc.vector.memset(ones_b, 1.0)
    ctxw = const.tile([128, 1], F32)
    nc.sync.dma_start(ctxw, ctxw_in[:, :])
    qw = const.tile([128, 1], F32)
    nc.sync.dma_start(qw, qw_in[:, :])
    cqw = const.tile([128, 1], F32)
    nc.sync.dma_start(cqw, cqw_in[:, :])

    # one-shot q loads: q (f32, [h, b, q]) and qT (bf16, [q, b, h])
    q_all = const.tile([128, nb, Q], F32)
    nc.sync.dma_start(q_all, q_in.rearrange("b h q -> h b q"))
    qT_all = const.tile([128, nb, H], BF16)
    nc.sync.dma_start(qT_all, qT_in.rearrange("b q h -> q b h"))

    # batched tiny ops: q_cs = q*cqw + ctxw for all nb batches in one op
    q_cs_all = const.tile([128, nb, Q], BF16)
    nc.vector.tensor_scalar(q_cs_all, q_all, cqw, ctxw, MULT, ADD)
    # s1[b][q] = sum_h q[h,q]*qw[h]: nb tiny matmuls, one psum tile, one evac
    s1_tile = psA.tile([128, 512], F32, tag="psA")
    s1_ps = s1_tile[:, 0:nb]
    for b in range(nb):
        nc.tensor.matmul(s1_ps[:, b:b + 1], q_all[:, b, :], qw)
    s1_all = const.tile([128, nb], F32)
    nc.vector.tensor_copy(s1_all, s1_ps)

    c_tiles = []
    for b in range(min(2, nb)):
        c_sb = poolc.tile([128, C], BF16, tag="c_sb")
        nc.sync.dma_start(c_sb, c_in[b])
        c_tiles.append(c_sb)

    for b in range(nb):
        c_sb = c_tiles[b]
        if b + 2 < nb:
            nxt = poolc.tile([128, C], BF16, tag="c_sb")
            nc.sync.dma_start(nxt, c_in[b + 2])
            c_tiles.append(nxt)
        qT_bf = qT_all[:, b, :]
        q_cs = q_cs_all[:, b, :]

        # ---- misc PSUM bank: tmp cols 0..127; db col 128 ----
        misc = psM.tile([128, 129], F32, tag="misc")
        tmp_ps = misc[:, 0:128]
        db_ps = misc[:, 128:129]

        # ---- ET = exp(S^T) halves ----
        ET = big.tile([128, C], BF16, tag="ET")
        for h2 in range(2):
            sl = slice(512 * h2, 512 * (h2 + 1))
            st = psA.tile([128, 512], F32, tag="psA")
            nc.tensor.matmul(st, q_cs, c_sb[:, sl])
            nc.scalar.activation(ET[:, sl], st, EXP, bias=s1_all[:, b:b + 1])

        # ---- A_T = ET / colsum_q(ET) (normalized a_att^T) ----
        A_T = big.tile([128, C], BF16, tag="A_T")
        if USE_DIVIDE:
            for h2 in range(2):
                sl = slice(512 * h2, 512 * (h2 + 1))
                da = psA.tile([128, 512], F32, tag="psA")
                nc.tensor.matmul(da, ones_b, ET[:, sl])
                nc.vector.tensor_tensor(A_T[:, sl], ET[:, sl], da, DIV)
        else:
            recD = big.tile([128, C], BF16, tag="recD")
            for h2 in range(2):
                sl = slice(512 * h2, 512 * (h2 + 1))
                da = psA.tile([128, 512], F32, tag="psA")
                nc.tensor.matmul(da, ones_b, ET[:, sl])
                with nc.allow_low_precision("softmax recip in bf16"):
                    nc.vector.reciprocal(recD[:, sl], da)
            nc.vector.tensor_mul(A_T, ET, recD)

        # ---- Ec = transpose(ET) chunks (exp(S) in [c, q] layout) ----
        ecT = psT.tile([128, NCK, 128], BF16, tag="psT")
        for j in range(NCK):
            nc.tensor.transpose(ecT[:, j, :], ET[:, 128 * j:128 * (j + 1)],
                                ident_b)
        Ec = big.tile([128, NCK, 128], BF16, tag="Ec")
        nc.scalar.activation(Ec, ecT, COPY)

        # ---- cT = transpose(c) chunks + ones column (for db) ----
        ctT = psT.tile([128, NCK, 128], BF16, tag="psT")
        for j in range(NCK):
            nc.tensor.transpose(ctT[:, j, :], c_sb[:, 128 * j:128 * (j + 1)],
                                ident_b)
        cT = big.tile([128, NCK, 129], BF16, tag="cT")
        nc.vector.tensor_copy(cT[:, :, 0:128], ctT)
        nc.gpsimd.memset(cT[:, :, 128:129], 1.0)

        # ---- [tmp | db] = sum_j Ec_j^T @ [cT_j | 1] ----
        for j in range(NCK):
            nc.tensor.matmul(misc[:, 0:129], Ec[:, j, :], cT[:, j, :],
                             start=(j == 0), stop=(j == NCK - 1))
        rdb = small.tile([128, 1], F32, tag="rdb")
        nc.vector.reciprocal(rdb, db_ps)
        tmp2 = small.tile([128, 128], BF16, tag="tmp2")
        nc.scalar.activation(tmp2, tmp_ps, COPY, scale=rdb)

        # ---- aT = qT^T @ A_T (scalar evac) then ca = c*aT (Pool) ----
        outa = big.tile([128, C], BF16, tag="outa")
        for h2 in range(2):
            sl = slice(512 * h2, 512 * (h2 + 1))
            ap_ = psA.tile([128, 512], F32, tag="psA")
            nc.tensor.matmul(ap_, qT_bf, A_T[:, sl])
            nc.scalar.activation(outa[:, sl], ap_, COPY)
        nc.sync.dma_start(out_a[b], outa)
        occ = big.tile([128, 2, C], BF16, tag="occ")
        nc.gpsimd.tensor_mul(occ[:, 0, :], c_sb, outa)

        # ---- bT = tmp2^T @ A_T; cb = c*bT straight from PSUM (DVE) ----
        for h2 in range(2):
            sl = slice(512 * h2, 512 * (h2 + 1))
            bp = psA.tile([128, 512], F32, tag="psA")
            nc.tensor.matmul(bp, tmp2, A_T[:, sl])
            nc.vector.tensor_mul(occ[:, 1, sl], c_sb[:, sl], bp)
        nc.sync.dma_start(out_cc[b].rearrange("k h c -> h k c"), occ)


def build_nc(nb: int = NB) -> bass.Bass:
    nc = bacc.Bacc("TRN2", target_bir_lowering=False, debug=False)
    c_in = nc.declare_dram_parameter("c", [nb, H, C], BF16, isOutput=False)
    q_in = nc.declare_dram_parameter("q", [nb, H, Q], F32, isOutput=False)
    qT_in = nc.declare_dram_parameter("qT", [nb, Q, H], BF16, isOutput=False)
    ctxw = nc.declare_dram_parameter("ctxw", [H, 1], F32, isOutput=False)
    qw = nc.declare_dram_parameter("qw", [H, 1], F32, isOutput=False)
    cqw = nc.declare_dram_parameter("cqw", [H, 1], F32, isOutput=False)
    out_a = nc.declare_dram_parameter("out_a", [nb, H, C], BF16, isOutput=True)
    out_cc = nc.declare_dram_parameter("out_cc", [nb, 2, H, C], BF16,
                                       isOutput=True)
    with tile.TileContext(nc) as tc:
        with ExitStack() as ctx:
            _body(ctx, tc, c_in[:], q_in[:], qT_in[:], ctxw[:], qw[:], cqw[:],
                  out_a[:], out_cc[:], nb)
    nc.compile()
    return nc


_NC_CACHE: dict = {}


def _get_nc(nb: int) -> bass.Bass:
    if nb not in _NC_CACHE:
        _NC_CACHE[nb] = build_nc(nb)
    return _NC_CACHE[nb]


def make_in_maps(inputs: dict, ncores: int = NCORES):
    c = np.asarray(inputs["c"], dtype=np.float32)
    q = np.ascontiguousarray(np.asarray(inputs["q"], dtype=np.float32))
    ctxw = np.ascontiguousarray(
        np.asarray(inputs["context_weights"], np.float32).reshape(H, 1))
    qw = np.ascontiguousarray(
        np.asarray(inputs["query_weights"], np.float32).reshape(H, 1))
    cqw = np.ascontiguousarray(
        np.asarray(inputs["cq_weights"], np.float32).reshape(H, 1))
    c_bf = np.ascontiguousarray(c).astype(ml_dtypes.bfloat16)
    qT_bf = np.ascontiguousarray(np.swapaxes(q, 1, 2)).astype(
        ml_dtypes.bfloat16)
    nb = c.shape[0] // ncores
    return [
        {
            "c": c_bf[i * nb:(i + 1) * nb],
            "q": q[i * nb:(i + 1) * nb],
            "qT": qT_bf[i * nb:(i + 1) * nb],
            "ctxw": ctxw,
            "qw": qw,
            "cqw": cqw,
        }
        for i in range(ncores)
    ], nb


def assemble(inputs: dict, results) -> np.ndarray:
    """Gather per-core device results into the full (B, 4H, C) f32 output."""
    c = np.asarray(inputs["c"], np.float32)
    nb = c.shape[0] // NCORES
    out = np.empty((c.shape[0], 4 * H, C), np.float32)
    out[:, 0:H] = c  # identity block, exact
    for i in range(NCORES):
        sl = slice(i * nb, (i + 1) * nb)
        out[sl, H:2 * H] = np.asarray(results[i]["out_a"]).astype(np.float32)
        occ = np.asarray(results[i]["out_cc"]).astype(np.float32)
        out[sl, 2 * H:3 * H] = occ[:, 0]
        out[sl, 3 * H:4 * H] = occ[:, 1]
    return out


def kernel(**inputs) -> np.ndarray:
    in_maps, nb = make_in_maps(inputs)
    nc = _get_nc(nb)
    res = run_bass_kernel_spmd(nc, in_maps, list(range(NCORES)))
    return assemble(inputs, res.results)


# revision 7
# speedup vs baseline: 1.4532x; 1.1803x over previous
"""Trainium2 Bass kernel for nn_ContextQueryAttention (B=64, H=128, C=1024, Q=128).

Sharding: pure data-parallel over batch — 8 batches per NeuronCore, SPMD on 8
cores. Params (tiny H-vectors) replicated to every core.

Math (masks are all-ones; softmax shift invariance lets bias be dropped):
  S = s0[c] + s1[q] + s2[c,q],  s2 = (c*cqw)^T q  (contraction over H)
  Fold s0 into the score matmul:  q_cs[h,q] = q[h,q]*cqw[h] + ctxw[h]
    => q_cs^T @ c = s2^T + s0  (row-broadcast), so ET = exp(S^T + s1) fully.
  a_att = softmax_q(S):  A_T = ET / colsum_q(ET)            [q, C]
  a^T   = qT^T @ A_T                                        [h, C]
  Ec    = transpose(ET) = exp(S) chunks                     [c, q]
  [tmp | db] = sum_j Ec_j^T @ [cT_j | 1]  (db = colsum_c)   [q, h+1]
  tmp2  = tmp / db;  b^T = tmp2^T @ A_T                     [h, C]
  out rows = [c; a^T; c*a^T; c*b^T]  — the c block is an identity copy of the
  input and is assembled host-side during the gather; device emits the other
  three row-blocks in bf16.

Host pre-shards c in bf16 and also ships q transposed (bf16) so the device
needs no q transpose. All matmuls bf16 with f32 PSUM.
"""

import numpy as np
import ml_dtypes
from contextlib import ExitStack

import concourse.bass as bass
import concourse.bacc as bacc
import concourse.tile as tile
from concourse import mybir
from concourse.bass_utils import run_bass_kernel_spmd
from concourse.masks import make_identity

F32 = mybir.dt.float32
BF16 = mybir.dt.bfloat16
EXP = mybir.ActivationFunctionType.Exp
COPY = mybir.ActivationFunctionType.Copy
MULT = mybir.AluOpType.mult
ADD = mybir.AluOpType.add
DIV = mybir.AluOpType.divide

B, H, C, Q = 64, 128, 1024, 128
NCORES = 8
NB = B // NCORES  # batches per core
NCK = C // 128    # 8 column chunks of C

# normalize a_att via DVE tensor_tensor divide straight from PSUM; fallback
# is reciprocal into bf16 recD plus a 4x multiply
USE_DIVIDE = False


def _body(ctx: ExitStack, tc: tile.TileContext, c_in, q_in, qT_in, ctxw_in,
          qw_in, cqw_in, out_a, out_cc, nb: int):
    nc = tc.nc

    const = ctx.enter_context(tc.tile_pool(name="const", bufs=1))
    big = ctx.enter_context(tc.tile_pool(name="big", bufs=3))
    poolc = ctx.enter_context(tc.tile_pool(name="poolc", bufs=4))
    small = ctx.enter_context(tc.tile_pool(name="small", bufs=2))
    # PSUM (8 banks): psA 4 x [128,512]f32 + psT 2 x [128,8,128]bf16 + psM 2
    psA = ctx.enter_context(tc.tile_pool(name="psA", bufs=4, space="PSUM"))
    psT = ctx.enter_context(tc.tile_pool(name="psT", bufs=2, space="PSUM"))
    psM = ctx.enter_context(tc.tile_pool(name="psM", bufs=2, space="PSUM"))

    # --- per-core constants ---
    ident_b = const.tile([128, 128], BF16)
    make_identity(nc, ident_b)
    ones_b = const.tile([128, 128], BF16)
    nc.vector.memset(ones_b, 1.0)
    ctxw = const.tile([128, 1], F32)
    nc.sync.dma_start(ctxw, ctxw_in[:, :])
    qw = const.tile([128, 1], F32)
    nc.sync.dma_start(qw, qw_in[:, :])
    cqw = const.tile([128, 1], F32)
    nc.sync.dma_start(cqw, cqw_in[:, :])

    # one-shot q loads: q (f32, [h, b, q]) and qT (bf16, [q, b, h])
    q_all = const.tile([128, nb, Q], F32)
    nc.sync.dma_start(q_all, q_in.rearrange("b h q -> h b q"))
    qT_all = const.tile([128, nb, H], BF16)
    nc.sync.dma_start(qT_all, qT_in.rearrange("b q h -> q b h"))

    # batched tiny ops: q_cs = q*cqw + ctxw for all nb batches in one op
    q_cs_all = const.tile([128, nb, Q], BF16)
    nc.vector.tensor_scalar(q_cs_all, q_all, cqw, ctxw, MULT, ADD)
    # s1[b][q] = sum_h q[h,q]*qw[h]: nb tiny matmuls, one psum tile, one evac
    s1_tile = psA.tile([128, 512], F32, tag="psA")
    s1_ps = s1_tile[:, 0:nb]
    for b in range(nb):
        nc.tensor.matmul(s1_ps[:, b:b + 1], q_all[:, b, :], qw)
    s1_all = const.tile([128, nb], F32)
    nc.vector.tensor_copy(s1_all, s1_ps)

    c_tiles = []
    for b in range(min(2, nb)):
        c_sb = poolc.tile([128, C], BF16, tag="c_sb")
        nc.sync.dma_start(c_sb, c_in[b])
        c_tiles.append(c_sb)

    # software-pipelined: head(b) computes scores/exp/transposes/tmp; tail(b)
    # does the normalization-dependent matmuls, products and stores. tail(b-1)
    # is emitted interleaved with head(b) so in-order engine queues stay fed.
    state: dict[int, dict] = {}

    def head(b):
        c_sb = c_tiles[b]
        if b + 2 < nb:
            nxt = poolc.tile([128, C], BF16, tag="c_sb")
            nc.sync.dma_start(nxt, c_in[b + 2])
            c_tiles.append(nxt)
        q_cs = q_cs_all[:, b, :]

        # misc PSUM bank: tmp cols 0..127; db col 128
        misc = psM.tile([128, 129], F32, tag="misc")

        # ET = exp(S^T) halves
        ET = big.tile([128, C], BF16, tag="ET")
        for h2 in range(2):
            sl = slice(512 * h2, 512 * (h2 + 1))
            st = psA.tile([128, 512], F32, tag="psA")
            nc.tensor.matmul(st, q_cs, c_sb[:, sl])
            nc.scalar.activation(ET[:, sl], st, EXP, bias=s1_all[:, b:b + 1])

        # A_T = ET / colsum_q(ET) (normalized a_att^T)
        A_T = big.tile([128, C], BF16, tag="A_T")
        recD = big.tile([128, C], BF16, tag="recD")
        for h2 in range(2):
            sl = slice(512 * h2, 512 * (h2 + 1))
            da = psA.tile([128, 512], F32, tag="psA")
            nc.tensor.matmul(da, ones_b, ET[:, sl])
            with nc.allow_low_precision("softmax recip in bf16"):
                nc.vector.reciprocal(recD[:, sl], da)
        nc.vector.tensor_mul(A_T, ET, recD)

        # Ec = transpose(ET) chunks (exp(S) in [c, q] layout)
        ecT = psT.tile([128, NCK, 128], BF16, tag="psT")
        for j in range(NCK):
            nc.tensor.transpose(ecT[:, j, :], ET[:, 128 * j:128 * (j + 1)],
                                ident_b)
        Ec = big.tile([128, NCK, 128], BF16, tag="Ec")
        nc.scalar.activation(Ec, ecT, COPY)

        # cT = transpose(c) chunks + ones column (for db)
        ctT = psT.tile([128, NCK, 128], BF16, tag="psT")
        for j in range(NCK):
            nc.tensor.transpose(ctT[:, j, :], c_sb[:, 128 * j:128 * (j + 1)],
                                ident_b)
        cT = big.tile([128, NCK, 129], BF16, tag="cT")
        nc.vector.tensor_copy(cT[:, :, 0:128], ctT)
        nc.gpsimd.memset(cT[:, :, 128:129], 1.0)

        # [tmp | db] = sum_j Ec_j^T @ [cT_j | 1]
        for j in range(NCK):
            nc.tensor.matmul(misc[:, 0:129], Ec[:, j, :], cT[:, j, :],
                             start=(j == 0), stop=(j == NCK - 1))
        state[b] = {"misc": misc, "A_T": A_T, "c_sb": c_sb}

    def tail(b):
        st_ = state.pop(b)
        misc, A_T, c_sb = st_["misc"], st_["A_T"], st_["c_sb"]
        qT_bf = qT_all[:, b, :]

        rdb = small.tile([128, 1], F32, tag="rdb")
        nc.vector.reciprocal(rdb, misc[:, 128:129])
        tmp2 = small.tile([128, 128], BF16, tag="tmp2")
        nc.scalar.activation(tmp2, misc[:, 0:128], COPY, scale=rdb)

        # aT = qT^T @ A_T (scalar evac) then ca = c*aT (Pool)
        outa = big.tile([128, C], BF16, tag="outa")
        for h2 in range(2):
            sl = slice(512 * h2, 512 * (h2 + 1))
            ap_ = psA.tile([128, 512], F32, tag="psA")
            nc.tensor.matmul(ap_, qT_bf, A_T[:, sl])
            nc.scalar.activation(outa[:, sl], ap_, COPY)
        nc.sync.dma_start(out_a[b], outa)
        occ = big.tile([128, 2, C], BF16, tag="occ")
        nc.gpsimd.tensor_mul(occ[:, 0, :], c_sb, outa)

        # bT = tmp2^T @ A_T; cb = c*bT straight from PSUM (DVE)
        for h2 in range(2):
            sl = slice(512 * h2, 512 * (h2 + 1))
            bp = psA.tile([128, 512], F32, tag="psA")
            nc.tensor.matmul(bp, tmp2, A_T[:, sl])
            nc.vector.tensor_mul(occ[:, 1, sl], c_sb[:, sl], bp)
        nc.sync.dma_start(out_cc[b].rearrange("k h c -> h k c"), occ)

    for b in range(nb + 1):
        if b < nb:
            head(b)
        if b > 0:
            tail(b - 1)


def build_nc(nb: int = NB) -> bass.Bass:
    nc = bacc.Bacc("TRN2", target_bir_lowering=False, debug=False)
    c_in = nc.declare_dram_parameter("c", [nb, H, C], BF16, isOutput=False)
    q_in = nc.declare_dram_parameter("q", [nb, H, Q], F32, isOutput=False)
    qT_in = nc.declare_dram_parameter("qT", [nb, Q, H], BF16, isOutput=False)
    ctxw = nc.declare_dram_parameter("ctxw", [H, 1], F32, isOutput=False)
    qw = nc.declare_dram_parameter("qw", [H, 1], F32, isOutput=False)
    cqw = nc.declare_dram_parameter("cqw", [H, 1], F32, isOutput=False)
    out_a = nc.declare_dram_parameter("out_a", [nb, H, C], BF16, isOutput=True)
    out_cc = nc.declare_dram_parameter("out_cc", [nb, 2, H, C], BF16,
                                       isOutput=True)
    with tile.TileContext(nc) as tc:
        with ExitStack() as ctx:
            _body(ctx, tc, c_in[:], q_in[:], qT_in[:], ctxw[:], qw[:], cqw[:],
                  out_a[:], out_cc[:], nb)
    nc.compile()
    return nc


_NC_CACHE: dict = {}


def _get_nc(nb: int) -> bass.Bass:
    if nb not in _NC_CACHE:
        _NC_CACHE[nb] = build_nc(nb)
    return _NC_CACHE[nb]


def make_in_maps(inputs: dict, ncores: int = NCORES):
    c = np.asarray(inputs["c"], dtype=np.float32)
    q = np.ascontiguousarray(np.asarray(inputs["q"], dtype=np.float32))
    ctxw = np.ascontiguousarray(
        np.asarray(inputs["context_weights"], np.float32).reshape(H, 1))
    qw = np.ascontiguousarray(
        np.asarray(inputs["query_weights"], np.float32).reshape(H, 1))
    cqw = np.ascontiguousarray(
        np.asarray(inputs["cq_weights"], np.float32).reshape(H, 1))
    c_bf = np.ascontiguousarray(c).astype(ml_dtypes.bfloat16)
    qT_bf = np.ascontiguousarray(np.swapaxes(q, 1, 2)).astype(
        ml_dtypes.bfloat16)
    nb = c.shape[0] // ncores
    return [
        {
            "c": c_bf[i * nb:(i + 1) * nb],
            "q": q[i * nb:(i + 1) * nb],
            "qT": qT_bf[i * nb:(i + 1) * nb],
            "ctxw": ctxw,
            "qw": qw,
            "cqw": cqw,
        }
        for i in range(ncores)
    ], nb


def assemble(inputs: dict, results) -> np.ndarray:
    """Gather per-core device results into the full (B, 4H, C) f32 output."""
    c = np.asarray(inputs["c"], np.float32)
    nb = c.shape[0] // NCORES
    out = np.empty((c.shape[0], 4 * H, C), np.float32)
    out[:, 0:H] = c  # identity block, exact
    for i in range(NCORES):
        sl = slice(i * nb, (i + 1) * nb)
        out[sl, H:2 * H] = np.asarray(results[i]["out_a"]).astype(np.float32)
        occ = np.asarray(results[i]["out_cc"]).astype(np.float32)
        out[sl, 2 * H:3 * H] = occ[:, 0]
        out[sl, 3 * H:4 * H] = occ[:, 1]
    return out


def kernel(**inputs) -> np.ndarray:
    in_maps, nb = make_in_maps(inputs)
    nc = _get_nc(nb)
    res = run_bass_kernel_spmd(nc, in_maps, list(range(NCORES)))
    return assemble(inputs, res.results)
